# revision 89
# baseline (speedup 1.0000x reference)
"""MHA (RoPE + causal softmax attention + out-proj) on 8 NeuronCores.

Sharding: DP4 x TP2. Core c: batch b = c % 4, head-group g = c // 4
(8 heads per core). Each core computes a transposed partial output
outT = (y_local @ w_o_slice^T)^T in [D, L]; host sums the two head-group
partials per batch (fp16) and transposes back.

All matmuls fp16 x fp16 -> fp32 PSUM (fp16 runs at the same PE rate as
bf16 in the cost model but with 10-bit mantissas). Layout strategy:
  Phase A: qkv natural layout [L, comps] via out = xT_tile.T @ w_chunk.
           Chunk 0 runs d-outer over 7 PSUM accumulators so matmuls can
           chase the startup DMAs (issue order is tuned; chunk-0 weights
           are loaded in per-d slices interleaved with the x tiles).
           RoPE applied with strided free-dim APs straight out of PSUM;
           rotated q/k staged to DRAM scratch (fp16) for the transposed
           reload; v copied directly into SBUF group tiles (no DRAM).
           The RoPE working pools close after chunk 4 so attention for
           (heads 0-3, q-chunk 0) interleaves into chunk 5 (v47), hiding
           its exp-bound work under the last GEMM chunk.
  Phase B: per head, q/k loaded back transposed ([comps, L]) via DMA xbar
           transpose; scores computed transposed (k on partitions) so the
           attn weights are ready as the moving operand of attn@V.
           exp on ScalarE with the 1/sqrt(HD) scale fused. Causal handled
           exactly at 128-col granularity: fully-masked k-tiles skipped,
           diagonal tiles column-trimmed, one [128,128] triangular mask.
           Softmax denominator accumulated on DVE (copy+adds over the
           fp16 attn tiles) with a single all-ones matmul per (h, qc)
           for the partition reduction (keeps TensorE free). Denominator
           reduce + normalize are deferred one block so PE never waits
           on the DVE add-chain.
  Phase C: out-proj outT[e, q] = sum_d w_oT[d, e] * yT[d, q]. As soon as
           a q-chunk's 8 heads are normalized, its 16 out-proj tiles are
           queued and fed into later attention blocks as PE filler, so
           the Activation engine (exp) never starves the PE; heads 0-3
           come first in the contraction so the tail does not wait on
           the last head's softmax chain.
"""

import contextlib

import numpy as np

import concourse.tile as tile
import concourse.mybir as mybir
from concourse import bacc
from concourse.bass_utils import run_bass_kernel_spmd

F16 = np.float16
F32 = mybir.dt.float32
FP16 = mybir.dt.float16

B, L, D, H, HD = 4, 2048, 2048, 16, 128
NH = 8                      # heads per core
DL = NH * HD                # 1024 local head dims
ROPE_BASE = 10000.0
ALPHA = float(HD) ** -0.5

LT = L // 128               # 16 L-tiles
DT = D // 128               # 16 D(contract)-tiles
NCH = 6                     # qkv chunks of 512 comps: q03,k03,v03,q47,k47,v47
QC = L // 512               # 4 q-chunks of 512
KT = L // 128               # 16 k-tiles


def _chunk_kind(c):
    # chunk order: q(heads0-3), k(0-3), v(0-3), q(4-7), k(4-7), v(4-7)
    return ("q", "k", "v")[c % 3], c // 3


def build_program(la=5, scb=3):
    nc = bacc.Bacc("TRN2", target_bir_lowering=False, debug=False, num_devices=8)

    xT = nc.dram_tensor("xT", [D, L], FP16, kind="ExternalInput").ap()
    wqkvT = nc.dram_tensor("wqkvT", [D, 3 * DL], FP16, kind="ExternalInput").ap()
    woT = nc.dram_tensor("woT", [DL, L], FP16, kind="ExternalInput").ap()
    chalf = nc.dram_tensor("chalf", [L, 256], FP16, kind="ExternalInput").ap()
    shalf = nc.dram_tensor("shalf", [L, 256], FP16, kind="ExternalInput").ap()
    trimask = nc.dram_tensor("trimask", [128, 128], FP16, kind="ExternalInput").ap()
    outT = nc.dram_tensor("outT", [D, L], FP16, kind="ExternalOutput").ap()

    # DRAM staging for rotated q/k (natural layout) only; v stays in SBUF
    qrot = nc.dram_tensor("qrot", [L, DL], FP16, kind="Internal").ap()
    krot = nc.dram_tensor("krot", [L, DL], FP16, kind="Internal").ap()

    with tile.TileContext(nc) as tc, contextlib.ExitStack() as es:
        pr = es.enter_context(tc.tile_pool(name="pR", bufs=1, side="right"))
        pqk03 = es.enter_context(
            tc.tile_pool(name="pQK03", bufs=1, side="right"))
        qkts03 = []
        qkts47 = []
        yts = [None] * NH
        vts = []
        wos = []

        # ---------------- attention block emitter ----------------
        def attn_head_qc(pools, h, qc, pe_denom=False, dpool=None,
                         filler=None):
            """Emit attention for (head h, q-chunk qc); returns a finale
            closure (denominator reduce + normalize) the caller defers
            so PE never blocks on the DVE add-chain."""
            pss, pba, psy, psd, pbr = pools
            g, hl = h // 4, h % 4
            qt, kt = (qkts03 + qkts47)[h]
            nkt = 4 * qc + 4
            ypsum = psy.tile([128, 512], F32, name="ypsum", tag="yp")
            dacc = None
            if pe_denom:
                # accumulated on PE during the block; borrow a phase-C bank
                # (psD would WAR-deadlock against the deferred finales)
                dpsum = dpool()
            else:
                dpsum = psd.tile([128, 512], F32, name="dpsum", tag="dp")
                dacc = pbr.tile([128, 512], FP16, name="dacc", tag="dacc")
            ats = {}

            def emit_score(j):
                m = j - 4 * qc  # >= 0 on the diagonal block
                c0 = max(m, 0) * 128  # first valid within-chunk col
                sc = pss.tile([128, 512], F32, name="sc", tag="sc")
                nc.tensor.matmul(
                    sc[:, c0:], kt[:, j * 128:(j + 1) * 128],
                    qt[:, qc * 512 + c0:(qc + 1) * 512],
                    start=True, stop=True)
                at = pba.tile([128, 512], FP16, name="at", tag="at")
                nc.scalar.activation(
                    out=at[:, c0:], in_=sc[:, c0:],
                    func=mybir.ActivationFunctionType.Exp,
                    scale=ALPHA)
                if m >= 0:
                    nc.vector.tensor_mul(
                        at[:, c0:c0 + 128], at[:, c0:c0 + 128], mt)
                ats[j] = at

            def emit_dadd_at(j, at):
                m = j - 4 * qc
                c0 = max(m, 0) * 128
                if pe_denom:
                    nc.tensor.matmul(
                        dpsum[:, c0:], ones128, at[:, c0:],
                        start=(j == 0), stop=(j == nkt - 1),
                        skip_group_check=True)
                elif j == 0:
                    nc.vector.tensor_copy(out=dacc, in_=at)
                else:
                    nc.vector.tensor_add(
                        dacc[:, c0:], dacc[:, c0:], at[:, c0:])

            n_off = 4 * qc
            next_emit = 0

            def emit_upto(n):
                nonlocal next_emit
                while next_emit < n:
                    emit_score(next_emit)
                    next_emit += 1

            emit_upto(min(la, nkt))
            if filler is not None:
                # PE work between the first scores and the first attn@V
                # consume hides the exp latency at block start
                filler()
            # off-diagonal k-tiles: full-width attn@V
            for j in range(n_off):
                emit_upto(min(j + 1 + la, nkt))
                at = ats.pop(j)
                nc.tensor.matmul(
                    ypsum, vts[g][:, j, hl * 128:(hl + 1) * 128], at,
                    start=(j == 0), stop=False)
                emit_dadd_at(j, at)
                if filler is not None and j % 2 == 1:
                    filler()

            # diagonal block: make sure all 4 at tiles exist first
            emit_upto(nkt)
            if filler is not None:
                filler()
                filler()
            d_ats = [ats.pop(4 * qc + m) for m in range(4)]
            for m in range(4):
                emit_dadd_at(4 * qc + m, d_ats[m])
            for mq in range(4):
                for m in range(mq + 1):
                    nc.tensor.matmul(
                        ypsum[:, mq * 128:(mq + 1) * 128],
                        vts[g][:, 4 * qc + m, hl * 128:(hl + 1) * 128],
                        d_ats[m][:, mq * 128:(mq + 1) * 128],
                        start=(qc == 0 and m == 0), stop=(m == mq),
                        skip_group_check=True)

            def finale():
                if not pe_denom:
                    nc.tensor.matmul(dpsum, ones128, dacc,
                                     start=True, stop=True)
                rbs = pbr.tile([128, 512], FP16, name="rbs", tag="rbs")
                with nc.allow_low_precision("softmax recip fp16"):
                    nc.vector.reciprocal(out=rbs, in_=dpsum)
                nc.vector.tensor_mul(
                    yts[h][:, qc * 512:(qc + 1) * 512], ypsum, rbs)

            return finale

        with tc.tile_pool(name="pAx", bufs=1) as pax, \
             tc.tile_pool(name="pAw", bufs=3) as paw:
            es_rope = contextlib.ExitStack()
            pcs = es_rope.enter_context(tc.tile_pool(name="pCs", bufs=1))
            pat = es_rope.enter_context(tc.tile_pool(name="pAt", bufs=2))
            pao = es_rope.enter_context(tc.tile_pool(name="pAo", bufs=3))
            # ------- resident left-side tiles + startup DMA order -------
            # DMA issue order is everything here: interleave per-d weight
            # slices with the x tiles the first 7 psum groups need, so
            # the d-outer matmuls start ~3.3us in and chase the queue.
            wch0 = paw.tile([128, DT, 512], FP16, name="wch", tag="wch")
            xts = []
            c_sb = pcs.tile([128, LT, 256], FP16, name="c_sb", tag="c_sb")
            s_sb = pcs.tile([128, LT, 256], FP16, name="s_sb", tag="s_sb")
            for d in range(7):
                nc.sync.dma_start(
                    out=wch0[:, d, :],
                    in_=wqkvT[d * 128:(d + 1) * 128, 0:512])
                xt = pax.tile([128, L], FP16, name=f"xt{d}", tag=f"xt{d}")
                nc.sync.dma_start(out=xt, in_=xT[d * 128:(d + 1) * 128, :])
                xts.append(xt)
            for d in range(7, DT):
                nc.sync.dma_start(
                    out=wch0[:, d, :],
                    in_=wqkvT[d * 128:(d + 1) * 128, 0:512])
            nc.sync.dma_start(
                out=c_sb[:, 0:8, :],
                in_=chalf[0:1024, :].rearrange("(i p) g -> p i g", p=128))
            nc.sync.dma_start(
                out=s_sb[:, 0:8, :],
                in_=shalf[0:1024, :].rearrange("(i p) g -> p i g", p=128))
            for d in range(7, DT):
                xt = pax.tile([128, L], FP16, name=f"xt{d}", tag=f"xt{d}")
                nc.sync.dma_start(out=xt, in_=xT[d * 128:(d + 1) * 128, :])
                xts.append(xt)
            nc.sync.dma_start(
                out=c_sb[:, 8:, :],
                in_=chalf[1024:, :].rearrange("(i p) g -> p i g", p=128))
            nc.sync.dma_start(
                out=s_sb[:, 8:, :],
                in_=shalf[1024:, :].rearrange("(i p) g -> p i g", p=128))

            mt = pr.tile([128, 128], FP16, name="trimask_sb", tag="mask")
            nc.sync.dma_start(out=mt, in_=trimask)
            ones128 = pr.tile([128, 128], FP16, name="ones128", tag="oc")
            nc.vector.memset(ones128, 1.0)

            for g in range(2):
                vt = pr.tile([128, LT, 512], FP16, name=f"vt{g}",
                             tag=f"vt{g}")
                vts.append(vt)

            # ---------------- Phase A: QKV + RoPE ----------------
            def rope_evac(pnat, i, kind, grp):
                """Evacuate one [128,512] qkv psum tile."""
                if kind == "v":
                    nc.scalar.copy(out=vts[grp][:, i, :], in_=pnat)
                    return
                x1 = pnat[:, 0::2]
                x2 = pnat[:, 1::2]
                ct = c_sb[:, i, :]
                st = s_sb[:, i, :]
                t1 = pat.tile([128, 256], F32, name="t1", tag="t1")
                nc.vector.tensor_mul(t1, x1, ct)
                t2 = pat.tile([128, 256], F32, name="t2", tag="t2")
                nc.vector.tensor_mul(t2, x2, st)
                t3 = pat.tile([128, 256], F32, name="t3", tag="t3")
                nc.vector.tensor_mul(t3, x2, ct)
                t4 = pat.tile([128, 256], F32, name="t4", tag="t4")
                nc.vector.tensor_mul(t4, x1, st)
                ro = pao.tile([128, 512], FP16, name="ro", tag="ro")
                nc.vector.tensor_sub(ro[:, 0::2], t1, t2)
                nc.vector.tensor_add(ro[:, 1::2], t3, t4)
                dst = qrot if kind == "q" else krot
                nc.sync.dma_start(
                    out=dst[i * 128:(i + 1) * 128,
                            grp * 512:(grp + 1) * 512],
                    in_=ro)

            # PSUM: psAV (2 banks, for the v47 chunk that overlaps phase B
            # pools) opens BEFORE psA6 so the pools can close in LIFO order
            # with no transition barrier at chunk 5.
            psav = es.enter_context(
                tc.tile_pool(name="psAV", bufs=1, space="PSUM"))

            def pv_tile(slot):
                return psav.tile([128, 512], F32, name="pv",
                                 tag=f"pv{slot}")

            avn = [0]

            def av_tile():
                t = pv_tile(avn[0] % 2)
                avn[0] += 1
                return t

            with tc.tile_pool(name="psA6", bufs=1, space="PSUM") as psa6:
                def pn_tile(slot):
                    return psa6.tile([128, 512], F32, name="pn",
                                     tag=f"pn{slot}")

                def c0_tile(i):
                    # chunk 0 borrows the psAV banks too: 8-deep rotation
                    s = i % 8
                    return pn_tile(s) if s < 6 else pv_tile(s - 6)

                # chunk 0 (q03) first part: d-outer over 7 psum banks so
                # the matmuls chase the startup DMAs (bank 8 stays free
                # for i=7 so it needn't wait on any evacuation).
                pns = [c0_tile(ii) for ii in range(7)]
                for d in range(DT):
                    for ii in range(7):
                        nc.tensor.matmul(
                            pns[ii],
                            xts[d][:, ii * 128:(ii + 1) * 128],
                            wch0[:, d, :],
                            start=(d == 0), stop=(d == DT - 1))
                for ii in range(7):
                    rope_evac(pns[ii], ii, "q", 0)
                # chunk 0 rest: data resident, i-outer so the RoPE
                # evacuations drain while the matmuls continue.
                for i in range(7, LT):
                    pnat = c0_tile(i)
                    for d in range(DT):
                        nc.tensor.matmul(
                            pnat,
                            xts[d][:, i * 128:(i + 1) * 128],
                            wch0[:, d, :],
                            start=(d == 0), stop=(d == DT - 1))
                    rope_evac(pnat, i, "q", 0)

                # chunks 1-4: i-outer, d-inner (weights triple-buffered),
                # cycling through the 6 psA6 banks.
                for c in range(1, NCH - 1):
                    kind, grp = _chunk_kind(c)
                    wch = paw.tile([128, DT, 512], FP16, name="wch",
                                   tag="wch")
                    nc.sync.dma_start(
                        out=wch,
                        in_=wqkvT[:, c * 512:(c + 1) * 512].rearrange(
                            "(d p) e -> p d e", p=128))
                    for i in range(LT):
                        pnat = pn_tile(i % 6)
                        for d in range(DT):
                            nc.tensor.matmul(
                                pnat,
                                xts[d][:, i * 128:(i + 1) * 128],
                                wch[:, d, :],
                                start=(d == 0), stop=(d == DT - 1))
                        rope_evac(pnat, i, kind, grp)
                    if c == 1:
                        # rotated q/k for heads 0-3 are now staged in
                        # DRAM: issue their transposed reloads early so
                        # they drain during the rest of phase A.
                        for h in range(4):
                            qt = pqk03.tile([128, L], FP16, name="qt03",
                                            tag=f"qt{h}", bufs=1)
                            nc.sync.dma_start_transpose(
                                out=qt,
                                in_=qrot[:, h * 128:(h + 1) * 128])
                            kt = pqk03.tile([128, L], FP16, name="kt03",
                                            tag=f"kt{h}", bufs=1)
                            nc.sync.dma_start_transpose(
                                out=kt,
                                in_=krot[:, h * 128:(h + 1) * 128])
                            qkts03.append((qt, kt))

            # RoPE pools (cos/sin, temps, psA8) close here; open the
            # attention pools that must outlive phase A, then emit chunk 5
            # (v47, no RoPE) with (heads 0-3, qc 0) attention interleaved.
            es_rope.close()
            pba = es.enter_context(
                tc.tile_pool(name="pBa", bufs=10, side="right"))
            pbr = es.enter_context(
                tc.tile_pool(name="pBr", bufs=2, side="right"))
            pyts = es.enter_context(
                tc.tile_pool(name="pYts", bufs=1, side="right"))
            pss = es.enter_context(
                tc.tile_pool(name="psS", bufs=scb, space="PSUM"))
            psy = es.enter_context(
                tc.tile_pool(name="psY", bufs=2, space="PSUM"))
            psd = es.enter_context(
                tc.tile_pool(name="psD", bufs=1, space="PSUM"))
            for h in range(4):
                yts[h] = pyts.tile([128, L], FP16, name=f"yt{h}",
                                   tag=f"yt{h}")
            bpools = (pss, pba, psy, psd, pbr)
            pending = None
            c = NCH - 1
            kind, grp = _chunk_kind(c)
            wch = paw.tile([128, DT, 512], FP16, name="wch", tag="wch")
            nc.sync.dma_start(
                out=wch,
                in_=wqkvT[:, c * 512:(c + 1) * 512].rearrange(
                    "(d p) e -> p d e", p=128))
            inject = {4: (0, 0), 8: (0, 1), 12: (0, 2), 15: (0, 3)}
            for i in range(LT):
                pnat = pv_tile(i % 2)
                for d in range(DT):
                    nc.tensor.matmul(
                        pnat,
                        xts[d][:, i * 128:(i + 1) * 128],
                        wch[:, d, :],
                        start=(d == 0), stop=(d == DT - 1))
                rope_evac(pnat, i, kind, grp)
                if i in inject:
                    qci, hi = inject[i]
                    fin = attn_head_qc(bpools, hi, qci, dpool=av_tile)
                    if pending is not None:
                        pending()
                    pending = fin

        # ---------------- Phase B rest + Phase C ----------------
        pqkb = es.enter_context(
            tc.tile_pool(name="pQK47", bufs=1, side="right"))
        pcw = es.enter_context(tc.tile_pool(name="pCw", bufs=1))
        pco = es.enter_context(tc.tile_pool(name="pCo", bufs=4))
        for h in range(4, NH):
            qt = pqkb.tile([128, L], FP16, name="qt47", tag=f"qt{h}")
            nc.sync.dma_start_transpose(
                out=qt, in_=qrot[:, h * 128:(h + 1) * 128])
            kt = pqkb.tile([128, L], FP16, name="kt47", tag=f"kt{h}")
            nc.sync.dma_start_transpose(
                out=kt, in_=krot[:, h * 128:(h + 1) * 128])
            qkts47.append((qt, kt))
        for h in range(4, NH):
            yts[h] = pqkb.tile([128, L], FP16, name=f"yt{h}", tag=f"yt{h}")
        for dd in range(NH):
            wo = pcw.tile([128, L], FP16, name=f"wo{dd}", tag=f"wo{dd}")
            nc.sync.dma_start(out=wo, in_=woT[dd * 128:(dd + 1) * 128, :])
            wos.append(wo)

        # remaining blocks: interleave qc1 heads 0-3 (their q/k are
        # resident) with qc0 heads 4-7 so each block waits at most one
        # in-flight transposed reload.
        seq = [(1, 0), (1, 1), (0, 4), (0, 5), (0, 6), (0, 7),
               (1, 2), (1, 3), (1, 4), (1, 5), (1, 6), (1, 7)]
        seq += [(qc, h) for qc in (2, 3) for h in range(NH)]

        # phase C borrows the psAV banks (freed after chunk 5); pools on
        # the PSUM side must close LIFO so psAV simply stays open.
        def c_tile(qc, e):
            op = av_tile()
            for dd in range(NH):
                nc.tensor.matmul(
                    op,
                    wos[dd][:, e * 128:(e + 1) * 128],
                    yts[dd][:, qc * 512:(qc + 1) * 512],
                    start=(dd == 0), stop=(dd == NH - 1))
            ot = pco.tile([128, 512], FP16, name="ot", tag="ot")
            nc.scalar.copy(out=ot, in_=op)
            nc.sync.dma_start(
                out=outT[e * 128:(e + 1) * 128,
                         qc * 512:(qc + 1) * 512],
                in_=ot)

        cwork = []
        quota = [0]
        since_refill = [0]

        def filler():
            if cwork and quota[0] > 0:
                quota[0] -= 1
                cwork.pop(0)()

        emitted_fin = {0: 3, 1: 0, 2: 0, 3: 0}  # h0-h2 flushed in A
        for k, (qc, h) in enumerate(seq):
            since_refill[0] += 1
            bl = max(1, 8 - since_refill[0])
            quota[0] = (len(cwork) + bl - 1) // bl if cwork else 0
            last = k == len(seq) - 1
            fin = attn_head_qc(bpools, h, qc, pe_denom=last,
                               dpool=av_tile, filler=filler)
            if pending is not None:
                pending()
                fq = seq[k - 1] if k > 0 else (0, 3)
                emitted_fin[fq[0]] += 1
                if emitted_fin[fq[0]] == NH and fq[0] < QC - 1:
                    cwork.extend(
                        (lambda qq, ee: lambda: c_tile(qq, ee))(
                            fq[0], e) for e in range(DT))
                    since_refill[0] = 0
            if last:
                # all-PE denominator: the finale is cheap, flush it
                # now so phase C can be emitted last
                fin()
                pending = None
            else:
                pending = fin
        quota[0] = len(cwork)
        while cwork:
            filler()
        for e in range(DT):
            c_tile(QC - 1, e)
    nc.compile()
    return nc


_NC_CACHE = None


def _get_program():
    global _NC_CACHE
    if _NC_CACHE is None:
        _NC_CACHE = build_program()
    return _NC_CACHE


def _host_inputs(x, w_qkv, w_o):
    inv = 1.0 / (ROPE_BASE ** (np.arange(0, HD, 2, dtype=np.float64) / HD))
    ang = np.arange(L, dtype=np.float64)[:, None] * inv[None, :]
    chalf = np.tile(np.cos(ang), (1, 4)).astype(F16)          # [L, 256]
    shalf = np.tile(np.sin(ang), (1, 4)).astype(F16)
    p = np.arange(128)[:, None]
    f = np.arange(128)[None, :]
    trimask = (p <= f).astype(F16)                             # [128, 128]

    in_maps = []
    for c in range(8):
        b, g = c % 4, c // 4
        qr = w_qkv[g * DL:(g + 1) * DL]
        kr = w_qkv[D + g * DL:D + (g + 1) * DL]
        vr = w_qkv[2 * D + g * DL:2 * D + (g + 1) * DL]
        wqkvT = np.ascontiguousarray(
            np.concatenate([qr[:512], kr[:512], vr[:512],
                            qr[512:], kr[512:], vr[512:]], axis=0).T
        ).astype(F16)
        in_maps.append({
            "xT": np.ascontiguousarray(x[b].T).astype(F16),
            "wqkvT": wqkvT,
            "woT": np.ascontiguousarray(
                w_o[:, g * DL:(g + 1) * DL].T).astype(F16),
            "chalf": chalf,
            "shalf": shalf,
            "trimask": trimask,
        })
    return in_maps


def kernel(x, w_qkv, w_o, _trace=False):
    x = np.asarray(x, dtype=np.float32)
    w_qkv = np.asarray(w_qkv, dtype=np.float32)
    w_o = np.asarray(w_o, dtype=np.float32)
    nc = _get_program()
    in_maps = _host_inputs(x, w_qkv, w_o)
    res = run_bass_kernel_spmd(nc, in_maps, core_ids=list(range(8)),
                               trace=_trace)
    kernel.last_result = res
    parts = [r["outT"] for r in res.results]
    out = np.empty((B, L, D), dtype=np.float32)
    for b in range(B):
        out[b] = (parts[b].astype(np.float32) +
                  parts[b + 4].astype(np.float32)).T
    return out
